# revision 12
# baseline (speedup 1.0000x reference)
"""Trainium2 Bass kernel for nn_AdaptiveLinearWithChannel.

Reference computation (per channel c of 64):
    bias_idx[c] = int(t[0, c, 0] * 31)
    out[c]      = x[c] @ W[model_idx[c]] + B[bias_idx[c]]
with x [64, 2048, 256] f32, W [64, 256, 256] f32, B [32, 256] f32.

Sharding: channels split 8-per-core across 8 NeuronCores (pure expert/data
parallel, no cross-device traffic). The per-channel weight gather
(W[model_idx]) and bias gather (B[bias_idx]) happen host-side while
sharding, per the sharding hint. x is passed to each core pre-transposed
and pre-swizzled to the exact SBUF partition layout so every device DMA
is a fully contiguous block; the device computes out^T per channel and
the host unswizzles back to [n, d_out].
"""

import os

import numpy as np

_N_CORES = 8
_C = 64           # channels
_N = 2048         # points per channel
_DIN = 256
_DOUT = 256
_NFRAMES = 32
_CLOC = _C // _N_CORES  # 8 channels per core

# matmul input dtype: "f32" (exact, 4 cyc/row), "f32r" (fast fp32, 1 cyc/row),
# "bf16" (inputs rounded to bf16), "bf16o" (bf16 inputs AND bf16 output DMA)
_VARIANT = os.environ.get("KERNEL_VARIANT", "bf16o")

_compiled = {}
LAST_RESULTS = None  # test harness reads exec_time_ns off this


def _build(variant):
    import concourse.bacc as bacc
    import concourse.bass as bass
    import concourse.mybir as mybir
    import concourse.tile as tile

    f32 = mybir.dt.float32
    out_dt = f32
    if variant == "bf16":
        in_dt = mybir.dt.bfloat16
        mm_dt = mybir.dt.bfloat16
    elif variant == "bf16o":
        in_dt = mybir.dt.bfloat16
        mm_dt = mybir.dt.bfloat16
        out_dt = mybir.dt.bfloat16
    elif variant == "f32r":
        in_dt = mybir.dt.float32r
        mm_dt = mybir.dt.float32r
    else:
        in_dt = f32
        mm_dt = f32

    nc = bacc.Bacc("TRN2", target_bir_lowering=False, debug=False)

    # all tensors pre-swizzled host-side to [*, p=128, a=2, free] so each
    # DMA is one contiguous block per partition
    xT = nc.declare_dram_parameter("xT", [_CLOC, 128, 2, _N], in_dt, isOutput=False)
    Wg = nc.declare_dram_parameter("Wg", [_CLOC, 128, 2, _DOUT], in_dt, isOutput=False)
    bgT = nc.declare_dram_parameter("bgT", [128, 2 * _CLOC], f32, isOutput=False)
    out = nc.declare_dram_parameter("out", [_CLOC, 128, 2, _N], out_dt, isOutput=True)

    NB = _N // 512  # 4 n-blocks of 512 per channel

    with tile.TileContext(nc) as tc:
        with (
            tc.tile_pool(name="xpool", bufs=4) as xpool,
            tc.tile_pool(name="wpool", bufs=4) as wpool,
            tc.tile_pool(name="bpool", bufs=1) as bpool,
            tc.tile_pool(name="opool", bufs=4) as opool,
            tc.tile_pool(name="psum", bufs=8, space=bass.MemorySpace.PSUM) as pspool,
        ):
            bias = bpool.tile([128, 2 * _CLOC], f32)
            # gpsimd (SWDGE): keeps this 128-descriptor scatter off the
            # HWDGE rings so it doesn't delay the first big x DMA
            nc.gpsimd.dma_start(bias[:], bgT[:])

            for c in range(_CLOC):
                # x^T for this channel: [128, 2, n], contiguous 8 KB/partition
                xt = xpool.tile([128, 2, _N], in_dt)
                nc.sync.dma_start(xt[:], xT[c])
                # weights: [128, 2, d_out], contiguous 1 KB/partition
                wt = wpool.tile([128, 2, _DOUT], in_dt)
                nc.sync.dma_start(wt[:], Wg[c])

                ot = opool.tile([128, 2, _N], out_dt)
                for oc in range(2):
                    b_ap = bias[:, c * 2 + oc : c * 2 + oc + 1]
                    for nb in range(NB):
                        ps = pspool.tile([128, 512], f32)
                        lhs0 = wt[:, 0, oc * 128 : (oc + 1) * 128]
                        lhs1 = wt[:, 1, oc * 128 : (oc + 1) * 128]
                        rhs0 = xt[:, 0, nb * 512 : (nb + 1) * 512]
                        rhs1 = xt[:, 1, nb * 512 : (nb + 1) * 512]
                        if mm_dt != in_dt:
                            lhs0 = lhs0.bitcast(mm_dt)
                            lhs1 = lhs1.bitcast(mm_dt)
                            rhs0 = rhs0.bitcast(mm_dt)
                            rhs1 = rhs1.bitcast(mm_dt)
                        nc.tensor.matmul(ps[:], lhs0, rhs0, start=True, stop=False)
                        nc.tensor.matmul(ps[:], lhs1, rhs1, start=False, stop=True)
                        o_ap = ot[:, oc, nb * 512 : (nb + 1) * 512]
                        # bias-add fused into the PSUM->SBUF copy; alternate
                        # engines so neither becomes the bottleneck
                        if nb % 2 == 0:
                            nc.vector.tensor_scalar_add(o_ap, ps[:], b_ap)
                        else:
                            nc.scalar.activation(
                                o_ap,
                                ps[:],
                                mybir.ActivationFunctionType.Identity,
                                bias=b_ap,
                            )
                # output DMA on the other HWDGE ring (ACT) so issue
                # doesn't serialize behind the input loads on sync
                nc.scalar.dma_start(out[c], ot[:])

    nc.compile()
    return nc


def kernel(x, t, model_idx, W, B):
    global LAST_RESULTS
    from concourse.bass_utils import run_bass_kernel_spmd

    x = np.asarray(x, dtype=np.float32)
    t = np.asarray(t, dtype=np.float32)
    model_idx = np.asarray(model_idx)
    W = np.asarray(W, dtype=np.float32)
    B = np.asarray(B, dtype=np.float32)

    # host-side routing (index tensors stay integer)
    bias_idx = (t[0, :, 0] * np.float32(_NFRAMES - 1)).astype(np.int32)
    Wg = W[model_idx]   # [64, 256, 256] gathered per-channel weights
    bg = B[bias_idx]    # [64, 256] gathered per-channel biases

    variant = _VARIANT
    if variant in ("bf16", "bf16o"):
        import ml_dtypes

        dev_dt = ml_dtypes.bfloat16
    else:
        dev_dt = np.float32

    # swizzle to the device layout: [c, p, a, free] with contraction index
    # i = a*128 + p on SBUF partitions
    # x [64, n, i] -> xdev[c, p, a, n] = x[c, n, a*128+p]
    xdev = np.ascontiguousarray(
        x.reshape(_C, _N, 2, 128).transpose(0, 3, 2, 1).astype(dev_dt)
    )
    # Wg [64, i, o] -> wdev[c, p, a, o] = Wg[c, a*128+p, o]
    wdev = np.ascontiguousarray(
        Wg.reshape(_C, 2, 128, _DOUT).transpose(0, 2, 1, 3).astype(dev_dt)
    )

    if variant not in _compiled:
        _compiled[variant] = _build(variant)
    nc = _compiled[variant]

    in_maps = []
    for k in range(_N_CORES):
        sl = slice(k * _CLOC, (k + 1) * _CLOC)
        # bias laid out for the device: bgT[p, c*2+oc] = bg[c, oc*128+p]
        bgT = np.ascontiguousarray(
            bg[sl].reshape(_CLOC, 2, 128).transpose(2, 0, 1).reshape(128, 2 * _CLOC)
        )
        in_maps.append({"xT": xdev[sl], "Wg": wdev[sl], "bgT": bgT})

    res = run_bass_kernel_spmd(nc, in_maps, core_ids=list(range(_N_CORES)))
    LAST_RESULTS = res

    out = np.empty((_C, _N, _DOUT), dtype=np.float32)
    for k in range(_N_CORES):
        # device out [c, p, a, n] -> out[c, n, a*128+p]
        odev = np.asarray(res.results[k]["out"]).astype(np.float32)
        out[k * _CLOC : (k + 1) * _CLOC] = odev.transpose(0, 3, 2, 1).reshape(
            _CLOC, _N, _DOUT
        )
    return out


# revision 25
# speedup vs baseline: 1.0921x; 1.0921x over previous
"""Trainium2 Bass kernel for nn_AdaptiveLinearWithChannel.

Reference computation (per channel c of 64):
    bias_idx[c] = int(t[0, c, 0] * 31)
    out[c]      = x[c] @ W[model_idx[c]] + B[bias_idx[c]]
with x [64, 2048, 256] f32, W [64, 256, 256] f32, B [32, 256] f32.

Sharding: channels split 8-per-core across 8 NeuronCores (pure expert/data
parallel, no cross-device traffic). The per-channel weight gather
(W[model_idx]) and bias gather (B[bias_idx]) happen host-side while
sharding, per the sharding hint. x is passed to each core pre-transposed
and pre-swizzled to the exact SBUF partition layout so every device DMA
is a fully contiguous block; the device computes out^T per channel and
the host unswizzles back to [n, d_out].
"""

import os

import numpy as np

_N_CORES = 8
_C = 64           # channels
_N = 2048         # points per channel
_DIN = 256
_DOUT = 256
_NFRAMES = 32
_CLOC = _C // _N_CORES  # 8 channels per core

# matmul input dtype: "f32" (exact, 4 cyc/row), "f32r" (fast fp32, 1 cyc/row),
# "bf16" (inputs rounded to bf16), "bf16o" (bf16 inputs AND bf16 output DMA)
_VARIANT = os.environ.get("KERNEL_VARIANT", "bf16o")

_compiled = {}
LAST_RESULTS = None  # test harness reads exec_time_ns off this


def _build(variant, bufs=4, first_split=2):
    import concourse.bacc as bacc
    import concourse.bass as bass
    import concourse.mybir as mybir
    import concourse.tile as tile

    f32 = mybir.dt.float32
    out_dt = f32
    if variant == "bf16":
        in_dt = mybir.dt.bfloat16
        mm_dt = mybir.dt.bfloat16
    elif variant == "bf16o":
        in_dt = mybir.dt.bfloat16
        mm_dt = mybir.dt.bfloat16
        out_dt = mybir.dt.bfloat16
    elif variant == "f32r":
        in_dt = mybir.dt.float32r
        mm_dt = mybir.dt.float32r
    else:
        in_dt = f32
        mm_dt = f32

    # Lean epilogue: the stock TileContext epilogue is drain + all-engine
    # butterfly barrier + semaphore clears + second barrier (~8 us), which
    # exists so a loaded NEFF can be re-executed. run_bass_kernel_spmd
    # loads the NEFF fresh per call (sems re-initialized by the load), so a
    # single drain that waits for all completion sems is sufficient.
    tail_mode = os.environ.get("KERNEL_LEAN_TAIL", "0")
    if tail_mode != "0":
        from concourse.vector_clock import ScopedClock

        def _lean_drain_and_barrier(self, tick_clock, wait_clock):
            drain_inst = self.nc.sync.drain()
            wait_clock.add_sem_waits(
                drain_inst.ins, ScopedClock({None: tick_clock.global_clock})
            )
            popped = self.nc._tile_sem_poison_stack.pop()
            assert popped is self._sem_poison
            if tail_mode == "sem_only":
                self.nc.all_engine_barrier(sem_only=True)

        tile.TileContext._drain_and_barrier = _lean_drain_and_barrier

    nc = bacc.Bacc("TRN2", target_bir_lowering=False, debug=False)

    # all tensors pre-swizzled host-side to [*, p=128, a=2, free] so each
    # DMA is one contiguous block per partition
    xT = nc.declare_dram_parameter("xT", [_CLOC, 128, 2, _N], in_dt, isOutput=False)
    Wg = nc.declare_dram_parameter("Wg", [_CLOC, 128, 2, _DOUT], in_dt, isOutput=False)
    bgT = nc.declare_dram_parameter("bgT", [128, 2 * _CLOC], f32, isOutput=False)
    out = nc.declare_dram_parameter("out", [_CLOC, 128, 2, _N], out_dt, isOutput=True)

    NB = _N // 512  # 4 n-blocks of 512 per channel

    with tile.TileContext(nc) as tc:
        with (
            tc.tile_pool(name="xpool", bufs=bufs) as xpool,
            tc.tile_pool(name="wpool", bufs=bufs) as wpool,
            tc.tile_pool(name="bpool", bufs=1) as bpool,
            tc.tile_pool(name="opool", bufs=bufs) as opool,
            tc.tile_pool(name="psum", bufs=8, space=bass.MemorySpace.PSUM) as pspool,
        ):
            bias = bpool.tile([128, 2 * _CLOC], f32)
            # gpsimd (SWDGE): keeps this 128-descriptor scatter off the
            # HWDGE rings so it doesn't delay the first big x DMA
            nc.gpsimd.dma_start(bias[:], bgT[:])

            for c in range(_CLOC):
                # x^T for this channel: [128, 2, n], contiguous 8 KB/partition.
                # For the first channel, land it in two halves so the first
                # matmuls start ~1.5us earlier.
                in_eng = nc.sync
                out_eng = nc.scalar
                xt = xpool.tile([128, 2, _N], in_dt)
                if c == 0:
                    q = _N // first_split
                    for j in range(first_split):
                        in_eng.dma_start(
                            xt[:, :, j * q : (j + 1) * q],
                            xT[c, :, :, j * q : (j + 1) * q],
                        )
                else:
                    in_eng.dma_start(xt[:], xT[c])
                # weights: [128, 2, d_out], contiguous 1 KB/partition, on the
                # idle SWDGE ring to keep sync's HWDGE ring for x only
                wt = wpool.tile([128, 2, _DOUT], in_dt)
                nc.gpsimd.dma_start(wt[:], Wg[c])

                ot = opool.tile([128, 2, _N], out_dt)
                for oc in range(2):
                    b_ap = bias[:, c * 2 + oc : c * 2 + oc + 1]
                    for nb in range(NB):
                        ps = pspool.tile([128, 512], f32)
                        lhs0 = wt[:, 0, oc * 128 : (oc + 1) * 128]
                        lhs1 = wt[:, 1, oc * 128 : (oc + 1) * 128]
                        rhs0 = xt[:, 0, nb * 512 : (nb + 1) * 512]
                        rhs1 = xt[:, 1, nb * 512 : (nb + 1) * 512]
                        if mm_dt != in_dt:
                            lhs0 = lhs0.bitcast(mm_dt)
                            lhs1 = lhs1.bitcast(mm_dt)
                            rhs0 = rhs0.bitcast(mm_dt)
                            rhs1 = rhs1.bitcast(mm_dt)
                        nc.tensor.matmul(ps[:], lhs0, rhs0, start=True, stop=False)
                        nc.tensor.matmul(ps[:], lhs1, rhs1, start=False, stop=True)
                        o_ap = ot[:, oc, nb * 512 : (nb + 1) * 512]
                        # bias-add fused into the PSUM->SBUF copy; alternate
                        # engines so neither becomes the bottleneck
                        if nb % 2 == 0:
                            nc.vector.tensor_scalar_add(o_ap, ps[:], b_ap)
                        else:
                            nc.scalar.activation(
                                o_ap,
                                ps[:],
                                mybir.ActivationFunctionType.Identity,
                                bias=b_ap,
                            )
                # output DMA on the other HWDGE ring (ACT) so issue
                # doesn't serialize behind the input loads on sync.
                # Last channel: store in halves so the final drain is short.
                if c == _CLOC - 1:
                    out_eng.dma_start(out[c, :, 0, :], ot[:, 0, :])
                    out_eng.dma_start(out[c, :, 1, :], ot[:, 1, :])
                else:
                    out_eng.dma_start(out[c], ot[:])

    nc.compile()
    return nc


def kernel(x, t, model_idx, W, B):
    global LAST_RESULTS
    from concourse.bass_utils import run_bass_kernel_spmd

    x = np.asarray(x, dtype=np.float32)
    t = np.asarray(t, dtype=np.float32)
    model_idx = np.asarray(model_idx)
    W = np.asarray(W, dtype=np.float32)
    B = np.asarray(B, dtype=np.float32)

    # host-side routing (index tensors stay integer)
    bias_idx = (t[0, :, 0] * np.float32(_NFRAMES - 1)).astype(np.int32)
    Wg = W[model_idx]   # [64, 256, 256] gathered per-channel weights
    bg = B[bias_idx]    # [64, 256] gathered per-channel biases

    variant = _VARIANT
    if variant in ("bf16", "bf16o"):
        import ml_dtypes

        dev_dt = ml_dtypes.bfloat16
    else:
        dev_dt = np.float32

    # swizzle to the device layout: [c, p, a, free] with contraction index
    # i = a*128 + p on SBUF partitions
    # x [64, n, i] -> xdev[c, p, a, n] = x[c, n, a*128+p]
    xdev = np.ascontiguousarray(
        x.reshape(_C, _N, 2, 128).transpose(0, 3, 2, 1).astype(dev_dt)
    )
    # Wg [64, i, o] -> wdev[c, p, a, o] = Wg[c, a*128+p, o]
    wdev = np.ascontiguousarray(
        Wg.reshape(_C, 2, 128, _DOUT).transpose(0, 2, 1, 3).astype(dev_dt)
    )

    if variant not in _compiled:
        _compiled[variant] = _build(variant)
    nc = _compiled[variant]

    in_maps = []
    for k in range(_N_CORES):
        sl = slice(k * _CLOC, (k + 1) * _CLOC)
        # bias laid out for the device: bgT[p, c*2+oc] = bg[c, oc*128+p]
        bgT = np.ascontiguousarray(
            bg[sl].reshape(_CLOC, 2, 128).transpose(2, 0, 1).reshape(128, 2 * _CLOC)
        )
        in_maps.append({"xT": xdev[sl], "Wg": wdev[sl], "bgT": bgT})

    try:
        res = run_bass_kernel_spmd(nc, in_maps, core_ids=list(range(_N_CORES)))
    except Exception:
        # transient NRT/axon failures (e.g. NRT_EXEC_UNIT_UNRECOVERABLE)
        # have been observed to succeed on retry
        res = run_bass_kernel_spmd(nc, in_maps, core_ids=list(range(_N_CORES)))
    LAST_RESULTS = res

    out = np.empty((_C, _N, _DOUT), dtype=np.float32)
    for k in range(_N_CORES):
        # device out [c, p, a, n] -> out[c, n, a*128+p]
        odev = np.asarray(res.results[k]["out"]).astype(np.float32)
        out[k * _CLOC : (k + 1) * _CLOC] = odev.transpose(0, 3, 2, 1).reshape(
            _CLOC, _N, _DOUT
        )
    return out


# revision 28
# speedup vs baseline: 1.1670x; 1.0686x over previous
"""Trainium2 Bass kernel for nn_AdaptiveLinearWithChannel.

Reference computation (per channel c of 64):
    bias_idx[c] = int(t[0, c, 0] * 31)
    out[c]      = x[c] @ W[model_idx[c]] + B[bias_idx[c]]
with x [64, 2048, 256] f32, W [64, 256, 256] f32, B [32, 256] f32.

Sharding: channels split 8-per-core across 8 NeuronCores (pure expert/data
parallel, no cross-device traffic). The per-channel weight gather
(W[model_idx]) and bias gather (B[bias_idx]) happen host-side while
sharding, per the sharding hint. x is passed to each core pre-transposed
and pre-swizzled to the exact SBUF partition layout so every device DMA
is a fully contiguous block; the device computes out^T per channel and
the host unswizzles back to [n, d_out].
"""

import os

import numpy as np

_N_CORES = 8
_C = 64           # channels
_N = 2048         # points per channel
_DIN = 256
_DOUT = 256
_NFRAMES = 32
_CLOC = _C // _N_CORES  # 8 channels per core

# matmul input dtype: "f32" (exact, 4 cyc/row), "f32r" (fast fp32, 1 cyc/row),
# "bf16" (inputs rounded to bf16), "bf16o" (bf16 inputs AND bf16 output DMA)
_VARIANT = os.environ.get("KERNEL_VARIANT", "bf16o")

_compiled = {}
LAST_RESULTS = None  # test harness reads exec_time_ns off this


def _build(variant, bufs=4, first_split=2, last_split=False, oc_split=False):
    import concourse.bacc as bacc
    import concourse.bass as bass
    import concourse.mybir as mybir
    import concourse.tile as tile

    f32 = mybir.dt.float32
    out_dt = f32
    if variant == "bf16":
        in_dt = mybir.dt.bfloat16
        mm_dt = mybir.dt.bfloat16
    elif variant == "bf16o":
        in_dt = mybir.dt.bfloat16
        mm_dt = mybir.dt.bfloat16
        out_dt = mybir.dt.bfloat16
    elif variant == "f32r":
        in_dt = mybir.dt.float32r
        mm_dt = mybir.dt.float32r
    else:
        in_dt = f32
        mm_dt = f32

    # Lean epilogue: the stock TileContext epilogue is drain + all-engine
    # butterfly barrier + semaphore clears + second barrier (~8 us), which
    # exists so a loaded NEFF can be re-executed. run_bass_kernel_spmd
    # loads the NEFF fresh per call (sems re-initialized by the load), so a
    # single drain that waits for all completion sems is sufficient.
    tail_mode = os.environ.get("KERNEL_LEAN_TAIL", "0")
    if tail_mode != "0":
        from concourse.vector_clock import ScopedClock

        def _lean_drain_and_barrier(self, tick_clock, wait_clock):
            drain_inst = self.nc.sync.drain()
            wait_clock.add_sem_waits(
                drain_inst.ins, ScopedClock({None: tick_clock.global_clock})
            )
            popped = self.nc._tile_sem_poison_stack.pop()
            assert popped is self._sem_poison
            if tail_mode == "sem_only":
                self.nc.all_engine_barrier(sem_only=True)

        tile.TileContext._drain_and_barrier = _lean_drain_and_barrier

    nc = bacc.Bacc("TRN2", target_bir_lowering=False, debug=False)

    # all tensors pre-swizzled host-side to [*, p=128, a=2, free] so each
    # DMA is one contiguous block per partition
    xT = nc.declare_dram_parameter("xT", [_CLOC, 128, 2, _N], in_dt, isOutput=False)
    Wg = nc.declare_dram_parameter("Wg", [_CLOC, 128, 2, _DOUT], in_dt, isOutput=False)
    bgT = nc.declare_dram_parameter("bgT", [128, 2 * _CLOC], f32, isOutput=False)
    out = nc.declare_dram_parameter("out", [_CLOC, 128, 2, _N], out_dt, isOutput=True)

    NB = _N // 512  # 4 n-blocks of 512 per channel

    with tile.TileContext(nc) as tc:
        with (
            tc.tile_pool(name="xpool", bufs=bufs) as xpool,
            tc.tile_pool(name="wpool", bufs=bufs) as wpool,
            tc.tile_pool(name="bpool", bufs=1) as bpool,
            tc.tile_pool(name="opool", bufs=bufs) as opool,
            tc.tile_pool(name="psum", bufs=8, space=bass.MemorySpace.PSUM) as pspool,
        ):
            bias = bpool.tile([128, 2 * _CLOC], f32)
            # gpsimd (SWDGE): keeps this 128-descriptor scatter off the
            # HWDGE rings so it doesn't delay the first big x DMA
            nc.gpsimd.dma_start(bias[:], bgT[:])

            for c in range(_CLOC):
                # x^T for this channel: [128, 2, n], contiguous 8 KB/partition.
                # For the first channel, land it in two halves so the first
                # matmuls start ~1.5us earlier.
                in_eng = nc.sync
                out_eng = nc.scalar
                xt = xpool.tile([128, 2, _N], in_dt)
                nsplit = 1
                if c == 0:
                    nsplit = first_split
                elif c == _CLOC - 1 and last_split:
                    nsplit = 2
                if nsplit > 1:
                    q = _N // nsplit
                    for j in range(nsplit):
                        in_eng.dma_start(
                            xt[:, :, j * q : (j + 1) * q],
                            xT[c, :, :, j * q : (j + 1) * q],
                        )
                else:
                    in_eng.dma_start(xt[:], xT[c])
                # weights: [128, 2, d_out], contiguous 1 KB/partition, on the
                # idle SWDGE ring to keep sync's HWDGE ring for x only
                wt = wpool.tile([128, 2, _DOUT], in_dt)
                nc.gpsimd.dma_start(wt[:], Wg[c])

                ot = opool.tile([128, 2, _N], out_dt)
                for oc in range(2):
                    b_ap = bias[:, c * 2 + oc : c * 2 + oc + 1]
                    for nb in range(NB):
                        ps = pspool.tile([128, 512], f32)
                        lhs0 = wt[:, 0, oc * 128 : (oc + 1) * 128]
                        lhs1 = wt[:, 1, oc * 128 : (oc + 1) * 128]
                        rhs0 = xt[:, 0, nb * 512 : (nb + 1) * 512]
                        rhs1 = xt[:, 1, nb * 512 : (nb + 1) * 512]
                        if mm_dt != in_dt:
                            lhs0 = lhs0.bitcast(mm_dt)
                            lhs1 = lhs1.bitcast(mm_dt)
                            rhs0 = rhs0.bitcast(mm_dt)
                            rhs1 = rhs1.bitcast(mm_dt)
                        nc.tensor.matmul(ps[:], lhs0, rhs0, start=True, stop=False)
                        nc.tensor.matmul(ps[:], lhs1, rhs1, start=False, stop=True)
                        o_ap = ot[:, oc, nb * 512 : (nb + 1) * 512]
                        # bias-add fused into the PSUM->SBUF copy; alternate
                        # engines so neither becomes the bottleneck
                        if nb % 2 == 0:
                            nc.vector.tensor_scalar_add(o_ap, ps[:], b_ap)
                        else:
                            nc.scalar.activation(
                                o_ap,
                                ps[:],
                                mybir.ActivationFunctionType.Identity,
                                bias=b_ap,
                            )
                # output DMA on the other HWDGE ring (ACT) so issue
                # doesn't serialize behind the input loads on sync.
                # Last channel: store in pieces so the final drain is short.
                if c == _CLOC - 1 and last_split:
                    h = _N // 2
                    for occ in range(2):
                        out_eng.dma_start(
                            out[c, :, occ, 0:h], ot[:, occ, 0:h]
                        )
                        out_eng.dma_start(
                            out[c, :, occ, h:_N], ot[:, occ, h:_N]
                        )
                elif c == _CLOC - 1 or oc_split:
                    out_eng.dma_start(out[c, :, 0, :], ot[:, 0, :])
                    out_eng.dma_start(out[c, :, 1, :], ot[:, 1, :])
                else:
                    out_eng.dma_start(out[c], ot[:])

    nc.compile()
    return nc


def kernel(x, t, model_idx, W, B):
    global LAST_RESULTS
    from concourse.bass_utils import run_bass_kernel_spmd

    x = np.asarray(x, dtype=np.float32)
    t = np.asarray(t, dtype=np.float32)
    model_idx = np.asarray(model_idx)
    W = np.asarray(W, dtype=np.float32)
    B = np.asarray(B, dtype=np.float32)

    # host-side routing (index tensors stay integer)
    bias_idx = (t[0, :, 0] * np.float32(_NFRAMES - 1)).astype(np.int32)
    Wg = W[model_idx]   # [64, 256, 256] gathered per-channel weights
    bg = B[bias_idx]    # [64, 256] gathered per-channel biases

    variant = _VARIANT
    if variant in ("bf16", "bf16o"):
        import ml_dtypes

        dev_dt = ml_dtypes.bfloat16
    else:
        dev_dt = np.float32

    # swizzle to the device layout: [c, p, a, free] with contraction index
    # i = a*128 + p on SBUF partitions
    # x [64, n, i] -> xdev[c, p, a, n] = x[c, n, a*128+p]
    xdev = np.ascontiguousarray(
        x.reshape(_C, _N, 2, 128).transpose(0, 3, 2, 1).astype(dev_dt)
    )
    # Wg [64, i, o] -> wdev[c, p, a, o] = Wg[c, a*128+p, o]
    wdev = np.ascontiguousarray(
        Wg.reshape(_C, 2, 128, _DOUT).transpose(0, 2, 1, 3).astype(dev_dt)
    )

    if variant not in _compiled:
        _compiled[variant] = _build(variant)
    nc = _compiled[variant]

    in_maps = []
    for k in range(_N_CORES):
        sl = slice(k * _CLOC, (k + 1) * _CLOC)
        # bias laid out for the device: bgT[p, c*2+oc] = bg[c, oc*128+p]
        bgT = np.ascontiguousarray(
            bg[sl].reshape(_CLOC, 2, 128).transpose(2, 0, 1).reshape(128, 2 * _CLOC)
        )
        in_maps.append({"xT": xdev[sl], "Wg": wdev[sl], "bgT": bgT})

    try:
        res = run_bass_kernel_spmd(nc, in_maps, core_ids=list(range(_N_CORES)))
    except Exception:
        # transient NRT/axon failures (e.g. NRT_EXEC_UNIT_UNRECOVERABLE)
        # have been observed to succeed on retry
        res = run_bass_kernel_spmd(nc, in_maps, core_ids=list(range(_N_CORES)))
    LAST_RESULTS = res

    out = np.empty((_C, _N, _DOUT), dtype=np.float32)
    for k in range(_N_CORES):
        # device out [c, p, a, n] -> out[c, n, a*128+p]
        odev = np.asarray(res.results[k]["out"]).astype(np.float32)
        out[k * _CLOC : (k + 1) * _CLOC] = odev.transpose(0, 3, 2, 1).reshape(
            _CLOC, _N, _DOUT
        )
    return out
